# revision 1
# baseline (speedup 1.0000x reference)
"""GCN (2-layer GCNConv + mean readout + sigmoid head) on 8 Trainium2 NeuronCores.

Strategy (graph/data parallel, dst-sharded):
  - Nodes are permuted (round-robin by in-degree) into NB = n_cores*nblk blocks of
    128 so every block has ~equal in-edge count; each core owns nblk blocks.
  - Per layer: H' = (D^-1/2 Z) @ W computed node-sharded on PE (bf16),
    AllGather of the bf16 feature table, then per dst-block: dma_gather of the
    source rows (table split in two halves so row ids fit int16), one-hot
    selection matrices (built on DVE via is_equal against an iota row)
    contracted on PE to form segment sums in PSUM. Self-loops are added via an
    identity-matrix matmul on the local shard.
  - dinv factors are separable: dinv_src is folded into the table rows,
    dinv_dst is applied post-aggregation (ACT activation scale).
  - Readout: per-block column sums via matmul against a pad-mask vector,
    accumulated in PSUM; final cross-core reduce + fc + sigmoid on host.
"""

import math

import numpy as np
import ml_dtypes

BF16 = ml_dtypes.bfloat16

# Problem constants (hardcoded per contract; kernel.py must be self-contained).
N = 50000
E = 800000
IN_DIM = 512
HID = 256
N_CORES = 8
P = 128
GB = 4  # dst-blocks per dma_gather instruction


def _wrap_idx(flat):
    """[L] int -> [128, L/16] int16 in the SWDGE wrapped layout."""
    L = len(flat)
    assert L % 16 == 0
    w = flat.reshape(L // 16, 16).T  # value i at [i%16, i//16]
    return np.ascontiguousarray(np.tile(w, (8, 1)).astype(np.int16))


# --------------------------------------------------------------------------- #
# Host-side preprocessing
# --------------------------------------------------------------------------- #

def _preprocess(x, edge_index, W1, b1, W2, b2):
    n, in_dim = x.shape
    hid = W1.shape[1]
    src = np.asarray(edge_index[0], dtype=np.int64)
    dst = np.asarray(edge_index[1], dtype=np.int64)

    deg_in = np.bincount(dst, minlength=n)
    deg = deg_in.astype(np.float64) + 1.0  # + self loop
    dinv = (1.0 / np.sqrt(deg)).astype(np.float32)

    nblk = math.ceil(n / (N_CORES * P))          # blocks per core
    NB = N_CORES * nblk                          # total blocks
    npad = NB * P
    nshard = nblk * P
    half = npad // 2
    assert half <= 32767, "table half must fit int16"

    # Balance blocks: deal nodes round-robin across blocks in desc in-degree
    # order -> every block gets ~equal total in-degree.
    order = np.argsort(-deg_in, kind="stable")
    i = np.arange(n)
    new_id = np.empty(n, dtype=np.int64)
    new_id[order] = (i % NB) * P + (i // NB)

    # Edge arrays in permuted space, sorted by (dst block, src half).
    s_new = new_id[src]
    d_new = new_id[dst]
    blk_id = d_new // P
    is_hi = (s_new >= half).astype(np.int64)
    skey = blk_id * 2 + is_hi
    eorder = np.argsort(skey, kind="stable")
    s_new = s_new[eorder]
    d_new = d_new[eorder]
    key_sorted = skey[eorder]

    cnt = np.bincount(key_sorted, minlength=2 * NB).reshape(NB, 2)
    c_lo = max(1, int(math.ceil(cnt[:, 0].max() / P)))
    c_hi = max(1, int(math.ceil(cnt[:, 1].max() / P)))
    c_tot = c_lo + c_hi

    # Per-(block, half) padded slots.
    idx_lo = np.zeros((NB, c_lo * P), dtype=np.int64)
    idx_hi = np.zeros((NB, c_hi * P), dtype=np.int64)
    dst_arr = np.full((NB, c_tot * P), -1.0, dtype=np.float32)

    starts = np.zeros(2 * NB + 1, dtype=np.int64)
    np.cumsum(cnt.reshape(-1), out=starts[1:])
    pos = np.arange(len(s_new)) - starts[key_sorted]
    lo_m = key_sorted % 2 == 0
    hi_m = ~lo_m
    b_lo, b_hi = key_sorted[lo_m] // 2, key_sorted[hi_m] // 2
    idx_lo[b_lo, pos[lo_m]] = s_new[lo_m]
    idx_hi[b_hi, pos[hi_m]] = s_new[hi_m] - half
    dst_arr[b_lo, pos[lo_m]] = (d_new[lo_m] % P).astype(np.float32)
    dst_arr[b_hi, c_lo * P + pos[hi_m]] = (d_new[hi_m] % P).astype(np.float32)

    dst_arr = dst_arr.reshape(NB, c_tot, P)

    # x' = dinv * x, permuted, padded, per-core transposed, bf16.
    xp = np.zeros((npad, in_dim), dtype=np.float32)
    xp[new_id] = x * dinv[:, None]

    dinv_pad = np.zeros(npad, dtype=np.float32)
    dinv_pad[new_id] = dinv
    mask_pad = np.zeros(npad, dtype=np.float32)
    mask_pad[new_id] = 1.0

    iota = np.broadcast_to(np.arange(P, dtype=np.float32), (P, P))
    ident = np.eye(P, dtype=np.float32)

    common = {
        "w1": np.ascontiguousarray(W1.astype(BF16)),
        "w2": np.ascontiguousarray(W2.astype(BF16)),
        "bias1": np.ascontiguousarray(np.broadcast_to(b1, (P, hid)).astype(np.float32)),
        "bias2": np.ascontiguousarray(np.broadcast_to(b2, (P, hid)).astype(np.float32)),
        "iota": np.ascontiguousarray(iota.astype(BF16)),
        "ident": np.ascontiguousarray(ident.astype(BF16)),
    }

    in_maps = []
    for c in range(N_CORES):
        lo_b, hi_b = c * nblk, (c + 1) * nblk
        lo_n, hi_n = c * nshard, (c + 1) * nshard
        m = dict(common)
        m["xT"] = np.ascontiguousarray(xp[lo_n:hi_n].T.astype(BF16))
        m["idxlo"] = _wrap_idx(idx_lo[lo_b:hi_b].reshape(-1))
        m["idxhi"] = _wrap_idx(idx_hi[lo_b:hi_b].reshape(-1))
        # [nblk, c_tot, P] -> [P, nblk*c_tot]
        m["dstf"] = np.ascontiguousarray(
            dst_arr[lo_b:hi_b].transpose(2, 0, 1).reshape(P, nblk * c_tot).astype(BF16))
        m["dinv"] = np.ascontiguousarray(
            dinv_pad[lo_n:hi_n].reshape(nblk, P).T.astype(np.float32))
        m["maskc"] = np.ascontiguousarray(
            mask_pad[lo_n:hi_n].reshape(nblk, P).T.astype(BF16))
        in_maps.append(m)

    meta = dict(nblk=nblk, c_lo=c_lo, c_hi=c_hi, in_dim=in_dim, hid=hid, n=n)
    return in_maps, meta


# --------------------------------------------------------------------------- #
# Device program
# --------------------------------------------------------------------------- #

def _build_nc(nblk, c_lo, c_hi, in_dim, hid, debug=False, variant="full", repeat=1):
    from contextlib import ExitStack

    from concourse import bass, mybir, bacc
    import concourse.tile as tile

    dt = mybir.dt
    nshard = nblk * P
    npad = N_CORES * nshard
    half = npad // 2
    KIN = in_dim // P
    KH = hid // P
    c_tot = c_lo + c_hi

    nc = bacc.Bacc(None, target_bir_lowering=False, num_devices=N_CORES)

    xT = nc.dram_tensor("xT", [in_dim, nshard], dt.bfloat16, kind="ExternalInput")
    w1 = nc.dram_tensor("w1", [in_dim, hid], dt.bfloat16, kind="ExternalInput")
    w2 = nc.dram_tensor("w2", [hid, hid], dt.bfloat16, kind="ExternalInput")
    bias1 = nc.dram_tensor("bias1", [P, hid], dt.float32, kind="ExternalInput")
    bias2 = nc.dram_tensor("bias2", [P, hid], dt.float32, kind="ExternalInput")
    iota = nc.dram_tensor("iota", [P, P], dt.bfloat16, kind="ExternalInput")
    ident = nc.dram_tensor("ident", [P, P], dt.bfloat16, kind="ExternalInput")
    idxlo = nc.dram_tensor("idxlo", [P, nblk * c_lo * 8], dt.int16, kind="ExternalInput")
    idxhi = nc.dram_tensor("idxhi", [P, nblk * c_hi * 8], dt.int16, kind="ExternalInput")
    dstf = nc.dram_tensor("dstf", [P, nblk * c_tot], dt.bfloat16, kind="ExternalInput")
    dinv = nc.dram_tensor("dinv", [P, nblk], dt.float32, kind="ExternalInput")
    maskc = nc.dram_tensor("maskc", [P, nblk], dt.bfloat16, kind="ExternalInput")
    out = nc.dram_tensor("partial", [P, KH], dt.float32, kind="ExternalOutput")
    if debug:
        dbg_h1 = nc.dram_tensor("dbg_h1", [P, nblk * hid], dt.bfloat16, kind="ExternalOutput")
        dbg_table1 = nc.dram_tensor("dbg_table1", [npad, hid], dt.bfloat16, kind="ExternalOutput")
        dbg_mlo = nc.dram_tensor("dbg_mlo", [P, GB * c_lo * hid], dt.bfloat16, kind="ExternalOutput")
        dbg_st = nc.dram_tensor("dbg_st", [P, c_tot * P], dt.bfloat16, kind="ExternalOutput")
        dbg_z1 = nc.dram_tensor("dbg_z1", [nblk * P, hid], dt.bfloat16, kind="ExternalOutput")
        dbg_zT = nc.dram_tensor("dbg_zT", [P, KH * nblk * P], dt.bfloat16, kind="ExternalOutput")

    with tile.TileContext(nc) as tc, ExitStack() as ctx:
        const = ctx.enter_context(tc.tile_pool(name="const", bufs=1))
        persist = ctx.enter_context(tc.tile_pool(name="persist", bufs=1))
        lhsp = ctx.enter_context(tc.tile_pool(name="lhsp", bufs=8))
        msgp = ctx.enter_context(tc.tile_pool(name="msgp", bufs=2))
        stp = ctx.enter_context(tc.tile_pool(name="stp", bufs=3))
        postp = ctx.enter_context(tc.tile_pool(name="postp", bufs=4))
        zp = ctx.enter_context(tc.tile_pool(name="zp", bufs=3))
        ps_mm = ctx.enter_context(tc.tile_pool(name="ps_mm", bufs=2, space="PSUM"))
        ps_agg = ctx.enter_context(tc.tile_pool(name="ps_agg", bufs=2, space="PSUM"))
        ps_cs = ctx.enter_context(tc.tile_pool(name="ps_cs", bufs=1, space="PSUM"))
        dram = ctx.enter_context(tc.tile_pool(name="dram", bufs=1, space="DRAM"))

        # ---- persistent / constant tiles ----
        w1_sb = const.tile([P, KIN * hid], dt.bfloat16, tag="w1_sb")
        w2_sb = const.tile([P, KH * hid], dt.bfloat16, tag="w2_sb")
        bias1_sb = const.tile([P, hid], dt.float32, tag="bias1_sb")
        bias2_sb = const.tile([P, hid], dt.float32, tag="bias2_sb")
        iota_sb = const.tile([P, P], dt.bfloat16, tag="iota_sb")
        ident_sb = const.tile([P, P], dt.bfloat16, tag="ident_sb")
        idxlo_sb = const.tile([P, nblk * c_lo * 8], dt.int16, tag="idxlo_sb")
        idxhi_sb = const.tile([P, nblk * c_hi * 8], dt.int16, tag="idxhi_sb")
        dst_sb = const.tile([P, nblk * c_tot], dt.bfloat16, tag="dst_sb")
        dinv_sb = const.tile([P, nblk], dt.float32, tag="dinv_sb")
        mask_sb = const.tile([P, nblk], dt.bfloat16, tag="mask_sb")

        zT_sb = persist.tile([P, KH * nshard], dt.bfloat16, tag="zT_sb")
        h_sb = persist.tile([P, nblk * hid], dt.bfloat16, tag="h_sb")

        hshard_d = dram.tile([nshard, hid], dt.bfloat16, tag="hshard_d")
        table_d = dram.tile([npad, hid], dt.bfloat16, tag="table_d")
        z1_d = dram.tile([nshard, hid], dt.bfloat16, tag="z1_d")

        # ---- constant loads ----
        nc.sync.dma_start(
            out=w1_sb[:, :].rearrange("p (k f) -> p k f", k=KIN),
            in_=w1[:, :].rearrange("(k p) f -> p k f", p=P))
        nc.sync.dma_start(
            out=w2_sb[:, :].rearrange("p (k f) -> p k f", k=KH),
            in_=w2[:, :].rearrange("(k p) f -> p k f", p=P))
        nc.sync.dma_start(out=bias1_sb[:, :], in_=bias1[:, :])
        nc.sync.dma_start(out=bias2_sb[:, :], in_=bias2[:, :])
        nc.sync.dma_start(out=iota_sb[:, :], in_=iota[:, :])
        nc.sync.dma_start(out=ident_sb[:, :], in_=ident[:, :])
        nc.sync.dma_start(out=idxlo_sb[:, :], in_=idxlo[:, :])
        nc.sync.dma_start(out=idxhi_sb[:, :], in_=idxhi[:, :])
        nc.sync.dma_start(out=dst_sb[:, :], in_=dstf[:, :])
        nc.sync.dma_start(out=dinv_sb[:, :], in_=dinv[:, :])
        nc.sync.dma_start(out=mask_sb[:, :], in_=maskc[:, :])

        def dense_layer(lhs_src, w_sb, kc, scale_from_dinv):
            """h_sb[:, nb*hid:...] = scale * (Z @ W) per block (bf16).

            lhs_src(nb, k) -> SBUF AP [P, P] bf16 (fin chunk k, node block nb).
            """
            for nb in range(nblk):
                ps = ps_mm.tile([P, hid], dt.float32, tag="mm")
                for k in range(kc):
                    nc.tensor.matmul(
                        out=ps[:, :],
                        lhsT=lhs_src(nb, k),
                        rhs=w_sb[:, k * hid:(k + 1) * hid],
                        start=(k == 0), stop=(k == kc - 1))
                scale = dinv_sb[:, nb:nb + 1] if scale_from_dinv else 1.0
                nc.scalar.activation(
                    h_sb[:, nb * hid:(nb + 1) * hid], ps[:, :],
                    mybir.ActivationFunctionType.Copy, scale=scale)

        def distribute():
            nc.sync.dma_start(
                out=hshard_d[:, :].rearrange("(nb p) f -> p nb f", p=P),
                in_=h_sb[:, :].rearrange("p (nb f) -> p nb f", nb=nblk))
            if variant == "nocoll":
                # timing bisection only: local copy instead of AllGather
                nc.sync.dma_start(out=table_d[0:nshard, :], in_=hshard_d[:, :])
                return
            nc.gpsimd.collective_compute(
                "AllGather", mybir.AluOpType.bypass,
                replica_groups=[list(range(N_CORES))],
                ins=[hshard_d[:, :].opt()],
                outs=[table_d[:, :].opt()])

        def aggregate(bias_sb, z_consumer, dbg=False):
            for g0 in range(0, nblk, GB):
                gb = min(GB, nblk - g0)
                mlo = msgp.tile([P, GB * c_lo * hid], dt.bfloat16, tag="mlo")
                mhi = msgp.tile([P, GB * c_hi * hid], dt.bfloat16, tag="mhi")
                if variant == "seqgather":
                    nc.sync.dma_start(
                        out=mlo[:, :gb * c_lo * hid]
                            .rearrange("p (c f) -> p c f", c=gb * c_lo),
                        in_=table_d[0:gb * c_lo * P, :]
                            .rearrange("(c p) f -> p c f", p=P))
                    nc.sync.dma_start(
                        out=mhi[:, :gb * c_hi * hid]
                            .rearrange("p (c f) -> p c f", c=gb * c_hi),
                        in_=table_d[half:half + gb * c_hi * P, :]
                            .rearrange("(c p) f -> p c f", p=P))
                elif variant == "noagg":
                    pass
                else:
                    nc.gpsimd.dma_gather(
                        out_ap=mlo[:, :gb * c_lo * hid]
                            .rearrange("p (c f) -> p c f", c=gb * c_lo),
                        in_ap=table_d[0:half, :],
                        idxs_ap=idxlo_sb[:, g0 * c_lo * 8:(g0 + gb) * c_lo * 8],
                        num_idxs=gb * c_lo * P,
                        num_idxs_reg=gb * c_lo * P,
                        elem_size=hid, single_packet=False)
                    nc.gpsimd.dma_gather(
                        out_ap=mhi[:, :gb * c_hi * hid]
                            .rearrange("p (c f) -> p c f", c=gb * c_hi),
                        in_ap=table_d[half:npad, :],
                        idxs_ap=idxhi_sb[:, g0 * c_hi * 8:(g0 + gb) * c_hi * 8],
                        num_idxs=gb * c_hi * P,
                        num_idxs_reg=gb * c_hi * P,
                        elem_size=hid, single_packet=False)
                if dbg and g0 == 0:
                    nc.sync.dma_start(out=dbg_mlo[:, :], in_=mlo[:, :])
                for bi in range(gb):
                    nb = g0 + bi
                    agg = ps_agg.tile([P, hid], dt.float32, tag="agg")
                    if variant != "noagg":
                        st = stp.tile([P, c_tot * P], dt.bfloat16, tag="st")
                        nc.vector.tensor_tensor(
                            out=st[:, :].rearrange("p (c q) -> p c q", c=c_tot),
                            in0=dst_sb[:, nb * c_tot:(nb + 1) * c_tot]
                                .unsqueeze(2).to_broadcast([P, c_tot, P]),
                            in1=iota_sb[:, :].unsqueeze(1).to_broadcast([P, c_tot, P]),
                            op=mybir.AluOpType.is_equal)
                        if dbg and nb == 0:
                            nc.sync.dma_start(out=dbg_st[:, :], in_=st[:, :])
                        for c in range(c_lo):
                            nc.tensor.matmul(
                                out=agg[:, :], lhsT=st[:, c * P:(c + 1) * P],
                                rhs=mlo[:, (bi * c_lo + c) * hid:(bi * c_lo + c + 1) * hid],
                                start=(c == 0), stop=False)
                        for c in range(c_hi):
                            nc.tensor.matmul(
                                out=agg[:, :], lhsT=st[:, (c_lo + c) * P:(c_lo + c + 1) * P],
                                rhs=mhi[:, (bi * c_hi + c) * hid:(bi * c_hi + c + 1) * hid],
                                start=False, stop=False)
                    nc.tensor.matmul(
                        out=agg[:, :], lhsT=ident_sb[:, :],
                        rhs=h_sb[:, nb * hid:(nb + 1) * hid],
                        start=(variant == "noagg"), stop=True)
                    v = postp.tile([P, hid], dt.float32, tag="v")
                    nc.scalar.activation(
                        v[:, :], agg[:, :], mybir.ActivationFunctionType.Copy,
                        scale=dinv_sb[:, nb:nb + 1])
                    w = postp.tile([P, hid], dt.float32, tag="w")
                    nc.vector.tensor_add(w[:, :], v[:, :], bias_sb[:, :])
                    z = zp.tile([P, hid], dt.bfloat16, tag="z")
                    nc.vector.tensor_scalar(
                        out=z[:, :], in0=w[:, :], scalar1=0.0, scalar2=None,
                        op0=mybir.AluOpType.max)
                    z_consumer(nb, z)

        # ================= layer 1 =================
        def xT_lhs(nb, k):
            t = lhsp.tile([P, P], dt.bfloat16, tag="xTt")
            nc.sync.dma_start(
                out=t[:, :], in_=xT[k * P:(k + 1) * P, nb * P:(nb + 1) * P])
            return t[:, :]

        for _rep in range(repeat):
            dense_layer(xT_lhs, w1_sb, KIN, scale_from_dinv=False)
            distribute()
            if debug:
                nc.sync.dma_start(out=dbg_h1[:, :], in_=h_sb[:, :])
                nc.sync.dma_start(out=dbg_table1[:, :], in_=table_d[:, :])

            def z1_write(nb, z):
                nc.sync.dma_start(out=z1_d[nb * P:(nb + 1) * P, :], in_=z[:, :])

            aggregate(bias1_sb, z1_write, dbg=debug)

            # transpose z1 (DRAM roundtrip, xbar transpose)
            for h in range(KH):
                nc.sync.dma_start(
                    out=zT_sb[:, h * nshard:(h + 1) * nshard],
                    in_=z1_d[:, h * P:(h + 1) * P],
                    transpose=True)
            if debug:
                nc.sync.dma_start(out=dbg_z1[:, :], in_=z1_d[:, :])
                nc.sync.dma_start(out=dbg_zT[:, :], in_=zT_sb[:, :])

            # ================= layer 2 =================
            def zT_lhs(nb, k):
                return zT_sb[:, k * nshard + nb * P:k * nshard + (nb + 1) * P]

            dense_layer(zT_lhs, w2_sb, KH, scale_from_dinv=True)
            distribute()

            cs = [ps_cs.tile([P, 1], dt.float32, tag=f"cs{h}", name=f"cs{h}")
                  for h in range(KH)]

            def colsum(nb, z):
                for h in range(KH):
                    nc.tensor.matmul(
                        out=cs[h][:, :], lhsT=z[:, h * P:(h + 1) * P],
                        rhs=mask_sb[:, nb:nb + 1],
                        start=(nb == 0), stop=(nb == nblk - 1))

            aggregate(bias2_sb, colsum)

        out_sb = postp.tile([P, KH], dt.float32, tag="out_sb")
        for h in range(KH):
            nc.vector.tensor_copy(out=out_sb[:, h:h + 1], in_=cs[h][:, :])
        nc.sync.dma_start(out=out[:, :], in_=out_sb[:, :])

    nc.compile()
    return nc


# --------------------------------------------------------------------------- #
# Entry point
# --------------------------------------------------------------------------- #

_CACHE = {}


def _run(x, edge_index, W1, b1, W2, b2, trace=False, debug=False):
    from concourse.bass_utils import run_bass_kernel_spmd

    in_maps, meta = _preprocess(x, edge_index, W1, b1, W2, b2)
    key = (meta["nblk"], meta["c_lo"], meta["c_hi"], meta["in_dim"], meta["hid"], debug)
    if key not in _CACHE:
        _CACHE[key] = _build_nc(*key[:5], debug=debug)
    nc = _CACHE[key]
    res = run_bass_kernel_spmd(
        nc, in_maps, core_ids=list(range(N_CORES)), trace=trace)
    parts = [r["partial"] for r in res.results]  # each [P, KH] f32
    colsum = np.sum(np.stack(parts), axis=0)     # [P, KH]
    g = colsum.T.reshape(-1)                     # [hid], g[h*P+p] = colsum[p, h]
    return g / float(meta["n"]), res


def kernel(x, edge_index, W1, b1, W2, b2, Wfc, bfc):
    x = np.asarray(x, dtype=np.float32)
    g, _ = _run(x, edge_index, np.asarray(W1, np.float32), np.asarray(b1, np.float32),
                np.asarray(W2, np.float32), np.asarray(b2, np.float32))
    logits = g.astype(np.float32) @ np.asarray(Wfc, np.float32) + np.asarray(bfc, np.float32)
    return (1.0 / (1.0 + np.exp(-logits))).astype(np.float32)



# revision 5
# speedup vs baseline: 1.3568x; 1.3568x over previous
"""GCN (2-layer GCNConv + mean readout + sigmoid head) on 8 Trainium2 NeuronCores.

Strategy (graph/data parallel, dst-sharded):
  - Nodes are permuted (round-robin by in-degree) into NB = n_cores*nblk blocks of
    128 so every block has ~equal in-edge count; each core owns nblk blocks.
  - Per layer: H' = (D^-1/2 Z) @ W computed node-sharded on PE (bf16),
    AllGather of the bf16 feature table (Shared output, one-hop writes), then
    per dst-block: dma_gather of the source rows (table split in two halves so
    row ids fit int16), one-hot selection matrices (built on DVE via is_equal
    against an iota row) contracted on PE to form segment sums in PSUM.
    Self-loops are added via an identity-matrix matmul on the local shard;
    bias is added in PSUM via a K=1 matmul of (1/dinv) x bias_row so the
    post-op collapses to a single ACT Relu with per-partition dinv scale.
  - Gathers round-robin over 4 SWDGE queues: each queue's descriptor
    generation runs on its own Q7 core pair, 4x the single-queue rate.
  - dinv factors are separable: dinv_src is folded into the table rows,
    dinv_dst is applied post-aggregation (ACT activation scale).
  - Readout: per-block column sums via matmul against a pad-mask vector,
    accumulated in PSUM; final cross-core reduce + fc + sigmoid on host.
"""

import math

import numpy as np
import ml_dtypes

BF16 = ml_dtypes.bfloat16

# Problem constants (hardcoded per contract; kernel.py must be self-contained).
N = 50000
E = 800000
IN_DIM = 512
HID = 256
N_CORES = 8
P = 128
GB = 2        # dst-blocks per dma_gather instruction
NQ = 4        # SWDGE queues (gather desc-gen parallelism)
MSG_BUFS = 4  # gather destination buffering depth


def _wrap_idx(flat):
    """[L] int -> [128, L/16] int16 in the SWDGE wrapped layout."""
    L = len(flat)
    assert L % 16 == 0
    w = flat.reshape(L // 16, 16).T  # value i at [i%16, i//16]
    return np.ascontiguousarray(np.tile(w, (8, 1)).astype(np.int16))


# --------------------------------------------------------------------------- #
# Host-side preprocessing
# --------------------------------------------------------------------------- #

def _preprocess(x, edge_index, W1, b1, W2, b2):
    n, in_dim = x.shape
    hid = W1.shape[1]
    src = np.asarray(edge_index[0], dtype=np.int64)
    dst = np.asarray(edge_index[1], dtype=np.int64)

    deg_in = np.bincount(dst, minlength=n)
    deg = deg_in.astype(np.float64) + 1.0  # + self loop
    dinv = (1.0 / np.sqrt(deg)).astype(np.float32)

    nblk = math.ceil(n / (N_CORES * P))          # blocks per core
    NB = N_CORES * nblk                          # total blocks
    npad = NB * P
    nshard = nblk * P
    half = npad // 2
    assert half <= 32767, "table half must fit int16"

    # Balance blocks: deal nodes round-robin across blocks in desc in-degree
    # order -> every block gets ~equal total in-degree.
    order = np.argsort(-deg_in, kind="stable")
    i = np.arange(n)
    new_id = np.empty(n, dtype=np.int64)
    new_id[order] = (i % NB) * P + (i // NB)

    # Edge arrays in permuted space, sorted by (dst block, src half).
    s_new = new_id[src]
    d_new = new_id[dst]
    blk_id = d_new // P
    is_hi = (s_new >= half).astype(np.int64)
    skey = blk_id * 2 + is_hi
    eorder = np.argsort(skey, kind="stable")
    s_new = s_new[eorder]
    d_new = d_new[eorder]
    key_sorted = skey[eorder]

    cnt = np.bincount(key_sorted, minlength=2 * NB).reshape(NB, 2)
    c_lo = max(1, int(math.ceil(cnt[:, 0].max() / P)))
    c_hi = max(1, int(math.ceil(cnt[:, 1].max() / P)))
    c_tot = c_lo + c_hi

    # Per-(block, half) padded slots.
    idx_lo = np.zeros((NB, c_lo * P), dtype=np.int64)
    idx_hi = np.zeros((NB, c_hi * P), dtype=np.int64)
    dst_arr = np.full((NB, c_tot * P), -1.0, dtype=np.float32)

    starts = np.zeros(2 * NB + 1, dtype=np.int64)
    np.cumsum(cnt.reshape(-1), out=starts[1:])
    pos = np.arange(len(s_new)) - starts[key_sorted]
    lo_m = key_sorted % 2 == 0
    hi_m = ~lo_m
    b_lo, b_hi = key_sorted[lo_m] // 2, key_sorted[hi_m] // 2
    idx_lo[b_lo, pos[lo_m]] = s_new[lo_m]
    idx_hi[b_hi, pos[hi_m]] = s_new[hi_m] - half
    dst_arr[b_lo, pos[lo_m]] = (d_new[lo_m] % P).astype(np.float32)
    dst_arr[b_hi, c_lo * P + pos[hi_m]] = (d_new[hi_m] % P).astype(np.float32)

    dst_arr = dst_arr.reshape(NB, c_tot, P)

    # x' = dinv * x, permuted, padded, per-core transposed, bf16.
    xp = np.zeros((npad, in_dim), dtype=np.float32)
    xp[new_id] = x * dinv[:, None]

    dinv_pad = np.zeros(npad, dtype=np.float32)
    dinv_pad[new_id] = dinv
    binv_pad = np.zeros(npad, dtype=np.float32)
    binv_pad[new_id] = 1.0 / dinv
    mask_pad = np.zeros(npad, dtype=np.float32)
    mask_pad[new_id] = 1.0

    iota = np.broadcast_to(np.arange(P, dtype=np.float32), (P, P))
    ident = np.eye(P, dtype=np.float32)

    common = {
        "w1": np.ascontiguousarray(W1.astype(BF16)),
        "w2": np.ascontiguousarray(W2.astype(BF16)),
        "brow1": np.ascontiguousarray(b1.reshape(1, hid).astype(BF16)),
        "brow2": np.ascontiguousarray(b2.reshape(1, hid).astype(BF16)),
        "iota": np.ascontiguousarray(iota.astype(BF16)),
        "ident": np.ascontiguousarray(ident.astype(BF16)),
    }

    in_maps = []
    for c in range(N_CORES):
        lo_b, hi_b = c * nblk, (c + 1) * nblk
        lo_n, hi_n = c * nshard, (c + 1) * nshard
        m = dict(common)
        m["xT"] = np.ascontiguousarray(xp[lo_n:hi_n].T.astype(BF16))
        m["idxlo"] = _wrap_idx(idx_lo[lo_b:hi_b].reshape(-1))
        m["idxhi"] = _wrap_idx(idx_hi[lo_b:hi_b].reshape(-1))
        # [nblk, c_tot, P] -> [P, nblk*c_tot]
        m["dstf"] = np.ascontiguousarray(
            dst_arr[lo_b:hi_b].transpose(2, 0, 1).reshape(P, nblk * c_tot).astype(BF16))
        m["dinv"] = np.ascontiguousarray(
            dinv_pad[lo_n:hi_n].reshape(nblk, P).T.astype(np.float32))
        m["binv"] = np.ascontiguousarray(
            binv_pad[lo_n:hi_n].reshape(1, nshard).astype(BF16))
        m["maskc"] = np.ascontiguousarray(
            mask_pad[lo_n:hi_n].reshape(nblk, P).T.astype(BF16))
        in_maps.append(m)

    meta = dict(nblk=nblk, c_lo=c_lo, c_hi=c_hi, in_dim=in_dim, hid=hid, n=n)
    return in_maps, meta


# --------------------------------------------------------------------------- #
# Device program
# --------------------------------------------------------------------------- #

def _build_nc(nblk, c_lo, c_hi, in_dim, hid, variant="full"):
    from contextlib import ExitStack

    from concourse import bass, mybir, bacc
    import concourse.tile as tile

    dt = mybir.dt
    nshard = nblk * P
    npad = N_CORES * nshard
    half = npad // 2
    KIN = in_dim // P
    KH = hid // P
    c_tot = c_lo + c_hi

    nc = bacc.Bacc(None, target_bir_lowering=False, num_devices=N_CORES,
                   num_swdge_queues=NQ)

    xT = nc.dram_tensor("xT", [in_dim, nshard], dt.bfloat16, kind="ExternalInput")
    w1 = nc.dram_tensor("w1", [in_dim, hid], dt.bfloat16, kind="ExternalInput")
    w2 = nc.dram_tensor("w2", [hid, hid], dt.bfloat16, kind="ExternalInput")
    brow1 = nc.dram_tensor("brow1", [1, hid], dt.bfloat16, kind="ExternalInput")
    brow2 = nc.dram_tensor("brow2", [1, hid], dt.bfloat16, kind="ExternalInput")
    iota = nc.dram_tensor("iota", [P, P], dt.bfloat16, kind="ExternalInput")
    ident = nc.dram_tensor("ident", [P, P], dt.bfloat16, kind="ExternalInput")
    idxlo = nc.dram_tensor("idxlo", [P, nblk * c_lo * 8], dt.int16, kind="ExternalInput")
    idxhi = nc.dram_tensor("idxhi", [P, nblk * c_hi * 8], dt.int16, kind="ExternalInput")
    dstf = nc.dram_tensor("dstf", [P, nblk * c_tot], dt.bfloat16, kind="ExternalInput")
    dinv = nc.dram_tensor("dinv", [P, nblk], dt.float32, kind="ExternalInput")
    binv = nc.dram_tensor("binv", [1, nshard], dt.bfloat16, kind="ExternalInput")
    maskc = nc.dram_tensor("maskc", [P, nblk], dt.bfloat16, kind="ExternalInput")
    out = nc.dram_tensor("partial", [P, KH], dt.float32, kind="ExternalOutput")

    with tile.TileContext(nc) as tc, ExitStack() as ctx:
        const = ctx.enter_context(tc.tile_pool(name="const", bufs=1))
        persist = ctx.enter_context(tc.tile_pool(name="persist", bufs=1))
        lhsp = ctx.enter_context(tc.tile_pool(name="lhsp", bufs=8))
        msgp = ctx.enter_context(tc.tile_pool(name="msgp", bufs=MSG_BUFS))
        stp = ctx.enter_context(tc.tile_pool(name="stp", bufs=3))
        zp = ctx.enter_context(tc.tile_pool(name="zp", bufs=3))
        ps_mm = ctx.enter_context(tc.tile_pool(name="ps_mm", bufs=2, space="PSUM"))
        ps_agg = ctx.enter_context(tc.tile_pool(name="ps_agg", bufs=2, space="PSUM"))
        ps_cs = ctx.enter_context(tc.tile_pool(name="ps_cs", bufs=1, space="PSUM"))
        dram = ctx.enter_context(tc.tile_pool(name="dram", bufs=1, space="DRAM"))

        # ---- persistent / constant tiles ----
        w1_sb = const.tile([P, KIN * hid], dt.bfloat16, tag="w1_sb")
        w2_sb = const.tile([P, KH * hid], dt.bfloat16, tag="w2_sb")
        brow1_sb = const.tile([1, hid], dt.bfloat16, tag="brow1_sb")
        brow2_sb = const.tile([1, hid], dt.bfloat16, tag="brow2_sb")
        iota_sb = const.tile([P, P], dt.bfloat16, tag="iota_sb")
        ident_sb = const.tile([P, P], dt.bfloat16, tag="ident_sb")
        idxlo_sb = const.tile([P, nblk * c_lo * 8], dt.int16, tag="idxlo_sb")
        idxhi_sb = const.tile([P, nblk * c_hi * 8], dt.int16, tag="idxhi_sb")
        dst_sb = const.tile([P, nblk * c_tot], dt.bfloat16, tag="dst_sb")
        dinv_sb = const.tile([P, nblk], dt.float32, tag="dinv_sb")
        binv_sb = const.tile([1, nshard], dt.bfloat16, tag="binv_sb")
        mask_sb = const.tile([P, nblk], dt.bfloat16, tag="mask_sb")

        zT_sb = persist.tile([P, KH * nshard], dt.bfloat16, tag="zT_sb")
        h_sb = persist.tile([P, nblk * hid], dt.bfloat16, tag="h_sb")

        hshard_d = dram.tile([nshard, hid], dt.bfloat16, tag="hshard_d")
        # Shared AG outputs must each have a single writer: one per layer.
        table1_d = dram.tile([npad, hid], dt.bfloat16, tag="table1_d",
                             addr_space="Shared")
        table2_d = dram.tile([npad, hid], dt.bfloat16, tag="table2_d",
                             addr_space="Shared")
        z1_d = dram.tile([nshard, hid], dt.bfloat16, tag="z1_d")

        # ---- constant loads ----
        nc.sync.dma_start(
            out=w1_sb[:, :].rearrange("p (k f) -> p k f", k=KIN),
            in_=w1[:, :].rearrange("(k p) f -> p k f", p=P))
        nc.sync.dma_start(
            out=w2_sb[:, :].rearrange("p (k f) -> p k f", k=KH),
            in_=w2[:, :].rearrange("(k p) f -> p k f", p=P))
        nc.sync.dma_start(out=brow1_sb[:, :], in_=brow1[:, :])
        nc.sync.dma_start(out=brow2_sb[:, :], in_=brow2[:, :])
        nc.sync.dma_start(out=iota_sb[:, :], in_=iota[:, :])
        nc.sync.dma_start(out=ident_sb[:, :], in_=ident[:, :])
        nc.sync.dma_start(out=idxlo_sb[:, :], in_=idxlo[:, :])
        nc.sync.dma_start(out=idxhi_sb[:, :], in_=idxhi[:, :])
        nc.sync.dma_start(out=dst_sb[:, :], in_=dstf[:, :])
        nc.sync.dma_start(out=dinv_sb[:, :], in_=dinv[:, :])
        nc.sync.dma_start(out=binv_sb[:, :], in_=binv[:, :])
        nc.sync.dma_start(out=mask_sb[:, :], in_=maskc[:, :])

        qcnt = [0]

        def next_q():
            q = qcnt[0] % NQ
            qcnt[0] += 1
            return q

        def dense_layer(lhs_src, w_sb, kc, scale_from_dinv):
            """h_sb[:, nb*hid:...] = scale * (Z @ W) per block (bf16)."""
            for nb in range(nblk):
                ps = ps_mm.tile([P, hid], dt.float32, tag="mm")
                for k in range(kc):
                    nc.tensor.matmul(
                        out=ps[:, :],
                        lhsT=lhs_src(nb, k),
                        rhs=w_sb[:, k * hid:(k + 1) * hid],
                        start=(k == 0), stop=(k == kc - 1))
                scale = dinv_sb[:, nb:nb + 1] if scale_from_dinv else 1.0
                nc.scalar.activation(
                    h_sb[:, nb * hid:(nb + 1) * hid], ps[:, :],
                    mybir.ActivationFunctionType.Copy, scale=scale)

        def distribute(table_d):
            nc.sync.dma_start(
                out=hshard_d[:, :].rearrange("(nb p) f -> p nb f", p=P),
                in_=h_sb[:, :].rearrange("p (nb f) -> p nb f", nb=nblk))
            nc.gpsimd.collective_compute(
                "AllGather", mybir.AluOpType.bypass,
                replica_groups=[list(range(N_CORES))],
                ins=[hshard_d[:, :].opt()],
                outs=[table_d[:, :].opt()])

        def aggregate(table_d, brow_sb, z_consumer):
            for g0 in range(0, nblk, GB):
                gb = min(GB, nblk - g0)
                mlo = msgp.tile([P, GB * c_lo * hid], dt.bfloat16, tag="mlo")
                mhi = msgp.tile([P, GB * c_hi * hid], dt.bfloat16, tag="mhi")
                if variant == "noagg":
                    pass
                else:
                    nc.gpsimd.dma_gather(
                        out_ap=mlo[:, :gb * c_lo * hid]
                            .rearrange("p (c f) -> p c f", c=gb * c_lo),
                        in_ap=table_d[0:half, :],
                        idxs_ap=idxlo_sb[:, g0 * c_lo * 8:(g0 + gb) * c_lo * 8],
                        num_idxs=gb * c_lo * P,
                        num_idxs_reg=gb * c_lo * P,
                        elem_size=hid, single_packet=False,
                        queue_num=next_q())
                    nc.gpsimd.dma_gather(
                        out_ap=mhi[:, :gb * c_hi * hid]
                            .rearrange("p (c f) -> p c f", c=gb * c_hi),
                        in_ap=table_d[half:npad, :],
                        idxs_ap=idxhi_sb[:, g0 * c_hi * 8:(g0 + gb) * c_hi * 8],
                        num_idxs=gb * c_hi * P,
                        num_idxs_reg=gb * c_hi * P,
                        elem_size=hid, single_packet=False,
                        queue_num=next_q())
                for bi in range(gb):
                    nb = g0 + bi
                    agg = ps_agg.tile([P, hid], dt.float32, tag="agg")
                    if variant != "noagg":
                        st = stp.tile([P, c_tot * P], dt.bfloat16, tag="st")
                        nc.vector.tensor_tensor(
                            out=st[:, :].rearrange("p (c q) -> p c q", c=c_tot),
                            in0=dst_sb[:, nb * c_tot:(nb + 1) * c_tot]
                                .unsqueeze(2).to_broadcast([P, c_tot, P]),
                            in1=iota_sb[:, :].unsqueeze(1).to_broadcast([P, c_tot, P]),
                            op=mybir.AluOpType.is_equal)
                        for c in range(c_lo):
                            nc.tensor.matmul(
                                out=agg[:, :], lhsT=st[:, c * P:(c + 1) * P],
                                rhs=mlo[:, (bi * c_lo + c) * hid:(bi * c_lo + c + 1) * hid],
                                start=(c == 0), stop=False)
                        for c in range(c_hi):
                            nc.tensor.matmul(
                                out=agg[:, :], lhsT=st[:, (c_lo + c) * P:(c_lo + c + 1) * P],
                                rhs=mhi[:, (bi * c_hi + c) * hid:(bi * c_hi + c + 1) * hid],
                                start=False, stop=False)
                    # bias as K=1 rank-1 update: (1/dinv)_col x bias_row, so the
                    # final dinv scale reconstitutes a plain bias add.
                    nc.tensor.matmul(
                        out=agg[:, :],
                        lhsT=binv_sb[0:1, nb * P:(nb + 1) * P],
                        rhs=brow_sb[0:1, :],
                        start=(variant == "noagg"), stop=False)
                    nc.tensor.matmul(
                        out=agg[:, :], lhsT=ident_sb[:, :],
                        rhs=h_sb[:, nb * hid:(nb + 1) * hid],
                        start=False, stop=True)
                    z = zp.tile([P, hid], dt.bfloat16, tag="z")
                    nc.scalar.activation(
                        z[:, :], agg[:, :], mybir.ActivationFunctionType.Relu,
                        scale=dinv_sb[:, nb:nb + 1])
                    z_consumer(nb, z)

        # ================= layer 1 =================
        def xT_lhs(nb, k):
            t = lhsp.tile([P, P], dt.bfloat16, tag="xTt")
            nc.sync.dma_start(
                out=t[:, :], in_=xT[k * P:(k + 1) * P, nb * P:(nb + 1) * P])
            return t[:, :]

        dense_layer(xT_lhs, w1_sb, KIN, scale_from_dinv=False)
        distribute(table1_d)

        def z1_write(nb, z):
            nc.sync.dma_start(out=z1_d[nb * P:(nb + 1) * P, :], in_=z[:, :])
            # transpose this block immediately (xbar), overlapped with the
            # rest of the aggregation.
            for h in range(KH):
                nc.sync.dma_start(
                    out=zT_sb[:, h * nshard + nb * P:h * nshard + (nb + 1) * P],
                    in_=z1_d[nb * P:(nb + 1) * P, h * P:(h + 1) * P],
                    transpose=True)

        aggregate(table1_d, brow1_sb, z1_write)

        # ================= layer 2 =================
        def zT_lhs(nb, k):
            return zT_sb[:, k * nshard + nb * P:k * nshard + (nb + 1) * P]

        dense_layer(zT_lhs, w2_sb, KH, scale_from_dinv=True)
        distribute(table2_d)

        cs = [ps_cs.tile([P, 1], dt.float32, tag=f"cs{h}", name=f"cs{h}")
              for h in range(KH)]

        def colsum(nb, z):
            for h in range(KH):
                nc.tensor.matmul(
                    out=cs[h][:, :], lhsT=z[:, h * P:(h + 1) * P],
                    rhs=mask_sb[:, nb:nb + 1],
                    start=(nb == 0), stop=(nb == nblk - 1))

        aggregate(table2_d, brow2_sb, colsum)

        out_sb = zp.tile([P, KH], dt.float32, tag="out_sb")
        for h in range(KH):
            nc.vector.tensor_copy(out=out_sb[:, h:h + 1], in_=cs[h][:, :])
        nc.sync.dma_start(out=out[:, :], in_=out_sb[:, :])

    nc.compile()
    return nc


# --------------------------------------------------------------------------- #
# Entry point
# --------------------------------------------------------------------------- #

_CACHE = {}


def _run(x, edge_index, W1, b1, W2, b2, trace=False):
    from concourse.bass_utils import run_bass_kernel_spmd

    in_maps, meta = _preprocess(x, edge_index, W1, b1, W2, b2)
    key = (meta["nblk"], meta["c_lo"], meta["c_hi"], meta["in_dim"], meta["hid"])
    if key not in _CACHE:
        _CACHE[key] = _build_nc(*key)
    nc = _CACHE[key]
    res = run_bass_kernel_spmd(
        nc, in_maps, core_ids=list(range(N_CORES)), trace=trace)
    parts = [r["partial"] for r in res.results]  # each [P, KH] f32
    colsum = np.sum(np.stack(parts), axis=0)     # [P, KH]
    g = colsum.T.reshape(-1)                     # [hid], g[h*P+p] = colsum[p, h]
    return g / float(meta["n"]), res


def kernel(x, edge_index, W1, b1, W2, b2, Wfc, bfc):
    x = np.asarray(x, dtype=np.float32)
    g, _ = _run(x, edge_index, np.asarray(W1, np.float32), np.asarray(b1, np.float32),
                np.asarray(W2, np.float32), np.asarray(b2, np.float32))
    logits = g.astype(np.float32) @ np.asarray(Wfc, np.float32) + np.asarray(bfc, np.float32)
    return (1.0 / (1.0 + np.exp(-logits))).astype(np.float32)


# revision 7
# speedup vs baseline: 1.7915x; 1.3204x over previous
"""GCN (2-layer GCNConv + mean readout + sigmoid head) on 8 Trainium2 NeuronCores.

Strategy (graph/data parallel, dst-sharded):
  - Nodes are permuted (round-robin by in-degree) into NB = n_cores*nblk blocks of
    128 so every block has ~equal in-edge count; each core owns nblk blocks.
  - Per layer: H' = (D^-1/2 Z) @ W computed node-sharded on PE (bf16),
    AllGather of the bf16 feature table (Shared output, one-hop writes), then
    per dst-block: dma_gather of the source rows (table split in two halves so
    row ids fit int16), one-hot selection matrices (built on DVE via is_equal
    against an iota row) contracted on PE to form segment sums in PSUM.
    Self-loops are added via an identity-matrix matmul on the local shard;
    bias is added in PSUM via a K=1 matmul of (1/dinv) x bias_row so the
    post-op collapses to a single ACT Relu with per-partition dinv scale.
  - Gathers round-robin over 4 SWDGE queues: each queue's descriptor
    generation runs on its own Q7 core pair, 4x the single-queue rate.
  - dinv factors are separable: dinv_src is folded into the table rows,
    dinv_dst is applied post-aggregation (ACT activation scale).
  - Readout: per-block column sums via matmul against a pad-mask vector,
    accumulated in PSUM; final cross-core reduce + fc + sigmoid on host.
"""

import math

import numpy as np
import ml_dtypes

BF16 = ml_dtypes.bfloat16

# Problem constants (hardcoded per contract; kernel.py must be self-contained).
N = 50000
E = 800000
IN_DIM = 512
HID = 256
N_CORES = 8
P = 128
GB = 1        # dst-blocks per dma_gather instruction
NQ = 4        # SWDGE queues (gather desc-gen parallelism)
MSG_BUFS = 8  # gather destination buffering depth


def _wrap_idx(flat):
    """[L] int -> [128, L/16] int16 in the SWDGE wrapped layout."""
    L = len(flat)
    assert L % 16 == 0
    w = flat.reshape(L // 16, 16).T  # value i at [i%16, i//16]
    return np.ascontiguousarray(np.tile(w, (8, 1)).astype(np.int16))


# --------------------------------------------------------------------------- #
# Host-side preprocessing
# --------------------------------------------------------------------------- #

def _preprocess(x, edge_index, W1, b1, W2, b2):
    n, in_dim = x.shape
    hid = W1.shape[1]
    src = np.asarray(edge_index[0], dtype=np.int64)
    dst = np.asarray(edge_index[1], dtype=np.int64)

    deg_in = np.bincount(dst, minlength=n)
    deg = deg_in.astype(np.float64) + 1.0  # + self loop
    dinv = (1.0 / np.sqrt(deg)).astype(np.float32)

    nblk = math.ceil(n / (N_CORES * P))          # blocks per core
    NB = N_CORES * nblk                          # total blocks
    npad = NB * P
    nshard = nblk * P
    half = npad // 2
    assert half <= 32767, "table half must fit int16"

    # Balance blocks: deal nodes round-robin across blocks in desc in-degree
    # order -> every block gets ~equal total in-degree.
    order = np.argsort(-deg_in, kind="stable")
    i = np.arange(n)
    new_id = np.empty(n, dtype=np.int64)
    new_id[order] = (i % NB) * P + (i // NB)

    # Edge arrays in permuted space, sorted by (dst block, src half).
    s_new = new_id[src]
    d_new = new_id[dst]
    blk_id = d_new // P
    is_hi = (s_new >= half).astype(np.int64)
    skey = blk_id * 2 + is_hi
    eorder = np.argsort(skey, kind="stable")
    s_new = s_new[eorder]
    d_new = d_new[eorder]
    key_sorted = skey[eorder]

    cnt = np.bincount(key_sorted, minlength=2 * NB).reshape(NB, 2)
    c_lo = max(1, int(math.ceil(cnt[:, 0].max() / P)))
    c_hi = max(1, int(math.ceil(cnt[:, 1].max() / P)))
    c_tot = c_lo + c_hi

    # Per-(block, half) padded slots.
    idx_lo = np.zeros((NB, c_lo * P), dtype=np.int64)
    idx_hi = np.zeros((NB, c_hi * P), dtype=np.int64)
    dst_arr = np.full((NB, c_tot * P), -1.0, dtype=np.float32)

    starts = np.zeros(2 * NB + 1, dtype=np.int64)
    np.cumsum(cnt.reshape(-1), out=starts[1:])
    pos = np.arange(len(s_new)) - starts[key_sorted]
    lo_m = key_sorted % 2 == 0
    hi_m = ~lo_m
    b_lo, b_hi = key_sorted[lo_m] // 2, key_sorted[hi_m] // 2
    idx_lo[b_lo, pos[lo_m]] = s_new[lo_m]
    idx_hi[b_hi, pos[hi_m]] = s_new[hi_m] - half
    dst_arr[b_lo, pos[lo_m]] = (d_new[lo_m] % P).astype(np.float32)
    dst_arr[b_hi, c_lo * P + pos[hi_m]] = (d_new[hi_m] % P).astype(np.float32)

    dst_arr = dst_arr.reshape(NB, c_tot, P)

    # x' = dinv * x, permuted, padded, per-core transposed, bf16.
    xp = np.zeros((npad, in_dim), dtype=np.float32)
    xp[new_id] = x * dinv[:, None]

    dinv_pad = np.zeros(npad, dtype=np.float32)
    dinv_pad[new_id] = dinv
    binv_pad = np.zeros(npad, dtype=np.float32)
    binv_pad[new_id] = 1.0 / dinv
    mask_pad = np.zeros(npad, dtype=np.float32)
    mask_pad[new_id] = 1.0

    iota = np.broadcast_to(np.arange(P, dtype=np.float32), (P, P))
    ident = np.eye(P, dtype=np.float32)

    common = {
        "w1": np.ascontiguousarray(W1.astype(BF16)),
        "w2": np.ascontiguousarray(W2.astype(BF16)),
        "brow1": np.ascontiguousarray(b1.reshape(1, hid).astype(BF16)),
        "brow2": np.ascontiguousarray(b2.reshape(1, hid).astype(BF16)),
        "iota": np.ascontiguousarray(iota.astype(BF16)),
        "ident": np.ascontiguousarray(ident.astype(BF16)),
    }

    in_maps = []
    for c in range(N_CORES):
        lo_b, hi_b = c * nblk, (c + 1) * nblk
        lo_n, hi_n = c * nshard, (c + 1) * nshard
        m = dict(common)
        m["xT"] = np.ascontiguousarray(xp[lo_n:hi_n].T.astype(BF16))
        m["idxlo"] = _wrap_idx(idx_lo[lo_b:hi_b].reshape(-1))
        m["idxhi"] = _wrap_idx(idx_hi[lo_b:hi_b].reshape(-1))
        # [nblk, c_tot, P] -> [P, nblk*c_tot]
        m["dstf"] = np.ascontiguousarray(
            dst_arr[lo_b:hi_b].transpose(2, 0, 1).reshape(P, nblk * c_tot).astype(BF16))
        m["dinv"] = np.ascontiguousarray(
            dinv_pad[lo_n:hi_n].reshape(nblk, P).T.astype(np.float32))
        m["binv"] = np.ascontiguousarray(
            binv_pad[lo_n:hi_n].reshape(1, nshard).astype(BF16))
        m["maskc"] = np.ascontiguousarray(
            mask_pad[lo_n:hi_n].reshape(nblk, P).T.astype(BF16))
        in_maps.append(m)

    meta = dict(nblk=nblk, c_lo=c_lo, c_hi=c_hi, in_dim=in_dim, hid=hid, n=n)
    return in_maps, meta


# --------------------------------------------------------------------------- #
# Device program
# --------------------------------------------------------------------------- #

def _build_nc(nblk, c_lo, c_hi, in_dim, hid, variant="full"):
    from contextlib import ExitStack

    from concourse import bass, mybir, bacc
    import concourse.tile as tile

    dt = mybir.dt
    nshard = nblk * P
    npad = N_CORES * nshard
    half = npad // 2
    KIN = in_dim // P
    KH = hid // P
    c_tot = c_lo + c_hi

    nc = bacc.Bacc(None, target_bir_lowering=False, num_devices=N_CORES,
                   num_swdge_queues=NQ)

    xT = nc.dram_tensor("xT", [in_dim, nshard], dt.bfloat16, kind="ExternalInput")
    w1 = nc.dram_tensor("w1", [in_dim, hid], dt.bfloat16, kind="ExternalInput")
    w2 = nc.dram_tensor("w2", [hid, hid], dt.bfloat16, kind="ExternalInput")
    brow1 = nc.dram_tensor("brow1", [1, hid], dt.bfloat16, kind="ExternalInput")
    brow2 = nc.dram_tensor("brow2", [1, hid], dt.bfloat16, kind="ExternalInput")
    iota = nc.dram_tensor("iota", [P, P], dt.bfloat16, kind="ExternalInput")
    ident = nc.dram_tensor("ident", [P, P], dt.bfloat16, kind="ExternalInput")
    idxlo = nc.dram_tensor("idxlo", [P, nblk * c_lo * 8], dt.int16, kind="ExternalInput")
    idxhi = nc.dram_tensor("idxhi", [P, nblk * c_hi * 8], dt.int16, kind="ExternalInput")
    dstf = nc.dram_tensor("dstf", [P, nblk * c_tot], dt.bfloat16, kind="ExternalInput")
    dinv = nc.dram_tensor("dinv", [P, nblk], dt.float32, kind="ExternalInput")
    binv = nc.dram_tensor("binv", [1, nshard], dt.bfloat16, kind="ExternalInput")
    maskc = nc.dram_tensor("maskc", [P, nblk], dt.bfloat16, kind="ExternalInput")
    out = nc.dram_tensor("partial", [P, KH], dt.float32, kind="ExternalOutput")

    with tile.TileContext(nc) as tc, ExitStack() as ctx:
        const = ctx.enter_context(tc.tile_pool(name="const", bufs=1))
        persist = ctx.enter_context(tc.tile_pool(name="persist", bufs=1))
        lhsp = ctx.enter_context(tc.tile_pool(name="lhsp", bufs=8))
        msgp = ctx.enter_context(tc.tile_pool(name="msgp", bufs=MSG_BUFS))
        stp = ctx.enter_context(tc.tile_pool(name="stp", bufs=3))
        zp = ctx.enter_context(tc.tile_pool(name="zp", bufs=3))
        ps_mm = ctx.enter_context(tc.tile_pool(name="ps_mm", bufs=2, space="PSUM"))
        ps_agg = ctx.enter_context(tc.tile_pool(name="ps_agg", bufs=2, space="PSUM"))
        ps_cs = ctx.enter_context(tc.tile_pool(name="ps_cs", bufs=1, space="PSUM"))
        dram = ctx.enter_context(tc.tile_pool(name="dram", bufs=1, space="DRAM"))

        # ---- persistent / constant tiles ----
        w1_sb = const.tile([P, KIN * hid], dt.bfloat16, tag="w1_sb")
        w2_sb = const.tile([P, KH * hid], dt.bfloat16, tag="w2_sb")
        brow1_sb = const.tile([1, hid], dt.bfloat16, tag="brow1_sb")
        brow2_sb = const.tile([1, hid], dt.bfloat16, tag="brow2_sb")
        iota_sb = const.tile([P, P], dt.bfloat16, tag="iota_sb")
        ident_sb = const.tile([P, P], dt.bfloat16, tag="ident_sb")
        idxlo_sb = const.tile([P, nblk * c_lo * 8], dt.int16, tag="idxlo_sb")
        idxhi_sb = const.tile([P, nblk * c_hi * 8], dt.int16, tag="idxhi_sb")
        dst_sb = const.tile([P, nblk * c_tot], dt.bfloat16, tag="dst_sb")
        dinv_sb = const.tile([P, nblk], dt.float32, tag="dinv_sb")
        binv_sb = const.tile([1, nshard], dt.bfloat16, tag="binv_sb")
        mask_sb = const.tile([P, nblk], dt.bfloat16, tag="mask_sb")

        zT_sb = persist.tile([P, KH * nshard], dt.bfloat16, tag="zT_sb")
        h_sb = persist.tile([P, nblk * hid], dt.bfloat16, tag="h_sb")

        hshard_d = dram.tile([nshard, hid], dt.bfloat16, tag="hshard_d")
        # Shared AG outputs must each have a single writer: one per layer.
        table1_d = dram.tile([npad, hid], dt.bfloat16, tag="table1_d",
                             addr_space="Shared")
        table2_d = dram.tile([npad, hid], dt.bfloat16, tag="table2_d",
                             addr_space="Shared")
        z1_d = dram.tile([nshard, hid], dt.bfloat16, tag="z1_d")

        # ---- constant loads ----
        nc.sync.dma_start(
            out=w1_sb[:, :].rearrange("p (k f) -> p k f", k=KIN),
            in_=w1[:, :].rearrange("(k p) f -> p k f", p=P))
        nc.sync.dma_start(
            out=w2_sb[:, :].rearrange("p (k f) -> p k f", k=KH),
            in_=w2[:, :].rearrange("(k p) f -> p k f", p=P))
        nc.sync.dma_start(out=brow1_sb[:, :], in_=brow1[:, :])
        nc.sync.dma_start(out=brow2_sb[:, :], in_=brow2[:, :])
        nc.sync.dma_start(out=iota_sb[:, :], in_=iota[:, :])
        nc.sync.dma_start(out=ident_sb[:, :], in_=ident[:, :])
        nc.sync.dma_start(out=idxlo_sb[:, :], in_=idxlo[:, :])
        nc.sync.dma_start(out=idxhi_sb[:, :], in_=idxhi[:, :])
        nc.sync.dma_start(out=dst_sb[:, :], in_=dstf[:, :])
        nc.sync.dma_start(out=dinv_sb[:, :], in_=dinv[:, :])
        nc.sync.dma_start(out=binv_sb[:, :], in_=binv[:, :])
        nc.sync.dma_start(out=mask_sb[:, :], in_=maskc[:, :])

        qcnt = [0]

        def next_q():
            q = qcnt[0] % NQ
            qcnt[0] += 1
            return q

        def dense_layer(lhs_src, w_sb, kc, scale_from_dinv):
            """h_sb[:, nb*hid:...] = scale * (Z @ W) per block (bf16)."""
            for nb in range(nblk):
                ps = ps_mm.tile([P, hid], dt.float32, tag="mm")
                for k in range(kc):
                    nc.tensor.matmul(
                        out=ps[:, :],
                        lhsT=lhs_src(nb, k),
                        rhs=w_sb[:, k * hid:(k + 1) * hid],
                        start=(k == 0), stop=(k == kc - 1))
                scale = dinv_sb[:, nb:nb + 1] if scale_from_dinv else 1.0
                nc.scalar.activation(
                    h_sb[:, nb * hid:(nb + 1) * hid], ps[:, :],
                    mybir.ActivationFunctionType.Copy, scale=scale)

        def distribute(table_d):
            nc.sync.dma_start(
                out=hshard_d[:, :].rearrange("(nb p) f -> p nb f", p=P),
                in_=h_sb[:, :].rearrange("p (nb f) -> p nb f", nb=nblk))
            nc.gpsimd.collective_compute(
                "AllGather", mybir.AluOpType.bypass,
                replica_groups=[list(range(N_CORES))],
                ins=[hshard_d[:, :].opt()],
                outs=[table_d[:, :].opt()])

        def aggregate(table_d, brow_sb, z_consumer):
            for g0 in range(0, nblk, GB):
                gb = min(GB, nblk - g0)
                mlo = msgp.tile([P, GB * c_lo * hid], dt.bfloat16, tag="mlo")
                mhi = msgp.tile([P, GB * c_hi * hid], dt.bfloat16, tag="mhi")
                if variant == "noagg":
                    pass
                else:
                    nc.gpsimd.dma_gather(
                        out_ap=mlo[:, :gb * c_lo * hid]
                            .rearrange("p (c f) -> p c f", c=gb * c_lo),
                        in_ap=table_d[0:half, :],
                        idxs_ap=idxlo_sb[:, g0 * c_lo * 8:(g0 + gb) * c_lo * 8],
                        num_idxs=gb * c_lo * P,
                        num_idxs_reg=gb * c_lo * P,
                        elem_size=hid, single_packet=False,
                        queue_num=next_q())
                    nc.gpsimd.dma_gather(
                        out_ap=mhi[:, :gb * c_hi * hid]
                            .rearrange("p (c f) -> p c f", c=gb * c_hi),
                        in_ap=table_d[half:npad, :],
                        idxs_ap=idxhi_sb[:, g0 * c_hi * 8:(g0 + gb) * c_hi * 8],
                        num_idxs=gb * c_hi * P,
                        num_idxs_reg=gb * c_hi * P,
                        elem_size=hid, single_packet=False,
                        queue_num=next_q())
                for bi in range(gb):
                    nb = g0 + bi
                    agg = ps_agg.tile([P, hid], dt.float32, tag="agg")
                    if variant != "noagg":
                        st = stp.tile([P, c_tot * P], dt.bfloat16, tag="st")
                        nc.vector.tensor_tensor(
                            out=st[:, :].rearrange("p (c q) -> p c q", c=c_tot),
                            in0=dst_sb[:, nb * c_tot:(nb + 1) * c_tot]
                                .unsqueeze(2).to_broadcast([P, c_tot, P]),
                            in1=iota_sb[:, :].unsqueeze(1).to_broadcast([P, c_tot, P]),
                            op=mybir.AluOpType.is_equal)
                        for c in range(c_lo):
                            nc.tensor.matmul(
                                out=agg[:, :], lhsT=st[:, c * P:(c + 1) * P],
                                rhs=mlo[:, (bi * c_lo + c) * hid:(bi * c_lo + c + 1) * hid],
                                start=(c == 0), stop=False)
                        for c in range(c_hi):
                            nc.tensor.matmul(
                                out=agg[:, :], lhsT=st[:, (c_lo + c) * P:(c_lo + c + 1) * P],
                                rhs=mhi[:, (bi * c_hi + c) * hid:(bi * c_hi + c + 1) * hid],
                                start=False, stop=False)
                    # bias as K=1 rank-1 update: (1/dinv)_col x bias_row, so the
                    # final dinv scale reconstitutes a plain bias add.
                    nc.tensor.matmul(
                        out=agg[:, :],
                        lhsT=binv_sb[0:1, nb * P:(nb + 1) * P],
                        rhs=brow_sb[0:1, :],
                        start=(variant == "noagg"), stop=False)
                    nc.tensor.matmul(
                        out=agg[:, :], lhsT=ident_sb[:, :],
                        rhs=h_sb[:, nb * hid:(nb + 1) * hid],
                        start=False, stop=True)
                    z = zp.tile([P, hid], dt.bfloat16, tag="z")
                    nc.scalar.activation(
                        z[:, :], agg[:, :], mybir.ActivationFunctionType.Relu,
                        scale=dinv_sb[:, nb:nb + 1])
                    z_consumer(nb, z)

        # ================= layer 1 =================
        def xT_lhs(nb, k):
            t = lhsp.tile([P, P], dt.bfloat16, tag="xTt")
            nc.sync.dma_start(
                out=t[:, :], in_=xT[k * P:(k + 1) * P, nb * P:(nb + 1) * P])
            return t[:, :]

        dense_layer(xT_lhs, w1_sb, KIN, scale_from_dinv=False)
        distribute(table1_d)

        def z1_write(nb, z):
            nc.sync.dma_start(out=z1_d[nb * P:(nb + 1) * P, :], in_=z[:, :])

        aggregate(table1_d, brow1_sb, z1_write)

        # transpose z1 (DRAM roundtrip, xbar transpose) in one batch so the
        # per-block writes above don't serialize against transpose reads.
        for h in range(KH):
            nc.sync.dma_start(
                out=zT_sb[:, h * nshard:(h + 1) * nshard],
                in_=z1_d[:, h * P:(h + 1) * P],
                transpose=True)

        # ================= layer 2 =================
        def zT_lhs(nb, k):
            return zT_sb[:, k * nshard + nb * P:k * nshard + (nb + 1) * P]

        dense_layer(zT_lhs, w2_sb, KH, scale_from_dinv=True)
        distribute(table2_d)

        cs = [ps_cs.tile([P, 1], dt.float32, tag=f"cs{h}", name=f"cs{h}")
              for h in range(KH)]

        def colsum(nb, z):
            for h in range(KH):
                nc.tensor.matmul(
                    out=cs[h][:, :], lhsT=z[:, h * P:(h + 1) * P],
                    rhs=mask_sb[:, nb:nb + 1],
                    start=(nb == 0), stop=(nb == nblk - 1))

        aggregate(table2_d, brow2_sb, colsum)

        out_sb = zp.tile([P, KH], dt.float32, tag="out_sb")
        for h in range(KH):
            nc.vector.tensor_copy(out=out_sb[:, h:h + 1], in_=cs[h][:, :])
        nc.sync.dma_start(out=out[:, :], in_=out_sb[:, :])

    nc.compile()
    return nc


# --------------------------------------------------------------------------- #
# Entry point
# --------------------------------------------------------------------------- #

_CACHE = {}


def _run(x, edge_index, W1, b1, W2, b2, trace=False):
    from concourse.bass_utils import run_bass_kernel_spmd

    in_maps, meta = _preprocess(x, edge_index, W1, b1, W2, b2)
    key = (meta["nblk"], meta["c_lo"], meta["c_hi"], meta["in_dim"], meta["hid"])
    if key not in _CACHE:
        _CACHE[key] = _build_nc(*key)
    nc = _CACHE[key]
    res = run_bass_kernel_spmd(
        nc, in_maps, core_ids=list(range(N_CORES)), trace=trace)
    parts = [r["partial"] for r in res.results]  # each [P, KH] f32
    colsum = np.sum(np.stack(parts), axis=0)     # [P, KH]
    g = colsum.T.reshape(-1)                     # [hid], g[h*P+p] = colsum[p, h]
    return g / float(meta["n"]), res


def kernel(x, edge_index, W1, b1, W2, b2, Wfc, bfc):
    x = np.asarray(x, dtype=np.float32)
    g, _ = _run(x, edge_index, np.asarray(W1, np.float32), np.asarray(b1, np.float32),
                np.asarray(W2, np.float32), np.asarray(b2, np.float32))
    logits = g.astype(np.float32) @ np.asarray(Wfc, np.float32) + np.asarray(bfc, np.float32)
    return (1.0 / (1.0 + np.exp(-logits))).astype(np.float32)


# revision 9
# speedup vs baseline: 2.6371x; 1.4720x over previous
"""GCN (2-layer GCNConv + mean readout + sigmoid head) on 8 Trainium2 NeuronCores.

Strategy (graph/data parallel, dst-sharded):
  - Nodes are permuted (round-robin by in-degree) into NB = n_cores*nblk blocks of
    128 so every block has ~equal in-edge count; each core owns nblk blocks.
  - Per layer: H' = (D^-1/2 Z) @ W computed node-sharded on PE (bf16),
    AllGather of the bf16 feature table (Shared output, one-hop writes), then
    per dst-block: dma_gather of the source rows (table split in two halves so
    row ids fit int16), one-hot selection matrices (built on DVE via is_equal
    against an iota row) contracted on PE to form segment sums in PSUM.
    Self-loops are added via an identity-matrix matmul on the local shard;
    bias is added in PSUM via a K=1 matmul of (1/dinv) x bias_row so the
    post-op collapses to a single ACT Relu with per-partition dinv scale.
  - Gathers round-robin over 4 SWDGE queues: each queue's descriptor
    generation runs on its own Q7 core pair, 4x the single-queue rate.
  - dinv factors are separable: dinv_src is folded into the table rows,
    dinv_dst is applied post-aggregation (ACT activation scale).
  - Readout: per-block column sums via matmul against a pad-mask vector,
    accumulated in PSUM; final cross-core reduce + fc + sigmoid on host.
"""

import math

import numpy as np
import ml_dtypes

BF16 = ml_dtypes.bfloat16

# Problem constants (hardcoded per contract; kernel.py must be self-contained).
N = 50000
E = 800000
IN_DIM = 512
HID = 256
N_CORES = 8
P = 128
GB = 1         # dst-blocks per dma_gather instruction
NQ = 4         # SWDGE queues (gather desc-gen parallelism)
MSG_BUFS = 12  # gather destination buffering depth


def _wrap_idx(flat):
    """[L] int -> [128, L/16] int16 in the SWDGE wrapped layout."""
    L = len(flat)
    assert L % 16 == 0
    w = flat.reshape(L // 16, 16).T  # value i at [i%16, i//16]
    return np.ascontiguousarray(np.tile(w, (8, 1)).astype(np.int16))


# --------------------------------------------------------------------------- #
# Host-side preprocessing
# --------------------------------------------------------------------------- #

def _preprocess(x, edge_index, W1, b1, W2, b2):
    n, in_dim = x.shape
    hid = W1.shape[1]
    src = np.asarray(edge_index[0], dtype=np.int64)
    dst = np.asarray(edge_index[1], dtype=np.int64)

    deg_in = np.bincount(dst, minlength=n)
    deg = deg_in.astype(np.float64) + 1.0  # + self loop
    dinv = (1.0 / np.sqrt(deg)).astype(np.float32)

    nblk = math.ceil(n / (N_CORES * P))          # blocks per core
    NB = N_CORES * nblk                          # total blocks
    npad = NB * P
    nshard = nblk * P
    half = npad // 2
    assert half <= 32767, "table half must fit int16"

    # Balance blocks: deal nodes round-robin across blocks in desc in-degree
    # order -> every block gets ~equal total in-degree.
    order = np.argsort(-deg_in, kind="stable")
    i = np.arange(n)
    new_id = np.empty(n, dtype=np.int64)
    new_id[order] = (i % NB) * P + (i // NB)

    # Edge arrays in permuted space, sorted by (dst block, src half).
    s_new = new_id[src]
    d_new = new_id[dst]
    blk_id = d_new // P
    is_hi = (s_new >= half).astype(np.int64)
    skey = blk_id * 2 + is_hi
    eorder = np.argsort(skey, kind="stable")
    s_new = s_new[eorder]
    d_new = d_new[eorder]
    key_sorted = skey[eorder]

    cnt = np.bincount(key_sorted, minlength=2 * NB).reshape(NB, 2)
    c_lo = max(1, int(math.ceil(cnt[:, 0].max() / P)))
    c_hi = max(1, int(math.ceil(cnt[:, 1].max() / P)))
    c_tot = c_lo + c_hi

    # Per-(block, half) padded slots.
    idx_lo = np.zeros((NB, c_lo * P), dtype=np.int64)
    idx_hi = np.zeros((NB, c_hi * P), dtype=np.int64)
    dst_arr = np.full((NB, c_tot * P), -1.0, dtype=np.float32)

    starts = np.zeros(2 * NB + 1, dtype=np.int64)
    np.cumsum(cnt.reshape(-1), out=starts[1:])
    pos = np.arange(len(s_new)) - starts[key_sorted]
    lo_m = key_sorted % 2 == 0
    hi_m = ~lo_m
    b_lo, b_hi = key_sorted[lo_m] // 2, key_sorted[hi_m] // 2
    idx_lo[b_lo, pos[lo_m]] = s_new[lo_m]
    idx_hi[b_hi, pos[hi_m]] = s_new[hi_m] - half
    dst_arr[b_lo, pos[lo_m]] = (d_new[lo_m] % P).astype(np.float32)
    dst_arr[b_hi, c_lo * P + pos[hi_m]] = (d_new[hi_m] % P).astype(np.float32)

    dst_arr = dst_arr.reshape(NB, c_tot, P)

    # x' = dinv * x, permuted, padded, per-core transposed, bf16.
    xp = np.zeros((npad, in_dim), dtype=np.float32)
    xp[new_id] = x * dinv[:, None]

    dinv_pad = np.zeros(npad, dtype=np.float32)
    dinv_pad[new_id] = dinv
    binv_pad = np.zeros(npad, dtype=np.float32)
    binv_pad[new_id] = 1.0 / dinv
    mask_pad = np.zeros(npad, dtype=np.float32)
    mask_pad[new_id] = 1.0

    iota = np.broadcast_to(np.arange(P, dtype=np.float32), (P, P))
    ident = np.eye(P, dtype=np.float32)

    common = {
        "w1": np.ascontiguousarray(W1.astype(BF16)),
        "w2": np.ascontiguousarray(W2.astype(BF16)),
        "brow1": np.ascontiguousarray(b1.reshape(1, hid).astype(BF16)),
        "brow2": np.ascontiguousarray(b2.reshape(1, hid).astype(BF16)),
        "iota": np.ascontiguousarray(iota.astype(BF16)),
        "ident": np.ascontiguousarray(ident.astype(BF16)),
    }

    in_maps = []
    for c in range(N_CORES):
        lo_b, hi_b = c * nblk, (c + 1) * nblk
        lo_n, hi_n = c * nshard, (c + 1) * nshard
        m = dict(common)
        m["xT"] = np.ascontiguousarray(xp[lo_n:hi_n].T.astype(BF16))
        m["idxlo"] = _wrap_idx(idx_lo[lo_b:hi_b].reshape(-1))
        m["idxhi"] = _wrap_idx(idx_hi[lo_b:hi_b].reshape(-1))
        # [nblk, c_tot, P] -> [P, nblk*c_tot]
        m["dstf"] = np.ascontiguousarray(
            dst_arr[lo_b:hi_b].transpose(2, 0, 1).reshape(P, nblk * c_tot).astype(BF16))
        m["dinv"] = np.ascontiguousarray(
            dinv_pad[lo_n:hi_n].reshape(nblk, P).T.astype(np.float32))
        m["binv"] = np.ascontiguousarray(
            binv_pad[lo_n:hi_n].reshape(1, nshard).astype(BF16))
        m["maskc"] = np.ascontiguousarray(
            mask_pad[lo_n:hi_n].reshape(nblk, P).T.astype(BF16))
        in_maps.append(m)

    meta = dict(nblk=nblk, c_lo=c_lo, c_hi=c_hi, in_dim=in_dim, hid=hid, n=n)
    return in_maps, meta


# --------------------------------------------------------------------------- #
# Device program
# --------------------------------------------------------------------------- #

def _build_nc(nblk, c_lo, c_hi, in_dim, hid, variant="full"):
    from contextlib import ExitStack

    from concourse import bass, mybir, bacc
    import concourse.tile as tile

    dt = mybir.dt
    nshard = nblk * P
    npad = N_CORES * nshard
    half = npad // 2
    KIN = in_dim // P
    KH = hid // P
    c_tot = c_lo + c_hi

    nc = bacc.Bacc(None, target_bir_lowering=False, num_devices=N_CORES,
                   num_swdge_queues=NQ)

    xT = nc.dram_tensor("xT", [in_dim, nshard], dt.bfloat16, kind="ExternalInput")
    w1 = nc.dram_tensor("w1", [in_dim, hid], dt.bfloat16, kind="ExternalInput")
    w2 = nc.dram_tensor("w2", [hid, hid], dt.bfloat16, kind="ExternalInput")
    brow1 = nc.dram_tensor("brow1", [1, hid], dt.bfloat16, kind="ExternalInput")
    brow2 = nc.dram_tensor("brow2", [1, hid], dt.bfloat16, kind="ExternalInput")
    iota = nc.dram_tensor("iota", [P, P], dt.bfloat16, kind="ExternalInput")
    ident = nc.dram_tensor("ident", [P, P], dt.bfloat16, kind="ExternalInput")
    idxlo = nc.dram_tensor("idxlo", [P, nblk * c_lo * 8], dt.int16, kind="ExternalInput")
    idxhi = nc.dram_tensor("idxhi", [P, nblk * c_hi * 8], dt.int16, kind="ExternalInput")
    dstf = nc.dram_tensor("dstf", [P, nblk * c_tot], dt.bfloat16, kind="ExternalInput")
    dinv = nc.dram_tensor("dinv", [P, nblk], dt.float32, kind="ExternalInput")
    binv = nc.dram_tensor("binv", [1, nshard], dt.bfloat16, kind="ExternalInput")
    maskc = nc.dram_tensor("maskc", [P, nblk], dt.bfloat16, kind="ExternalInput")
    out = nc.dram_tensor("partial", [P, KH], dt.float32, kind="ExternalOutput")

    with tile.TileContext(nc) as tc, ExitStack() as ctx:
        const = ctx.enter_context(tc.tile_pool(name="const", bufs=1))
        persist = ctx.enter_context(tc.tile_pool(name="persist", bufs=1))
        lhsp = ctx.enter_context(tc.tile_pool(name="lhsp", bufs=8))
        msgp = ctx.enter_context(tc.tile_pool(name="msgp", bufs=MSG_BUFS))
        stp = ctx.enter_context(tc.tile_pool(name="stp", bufs=3))
        zp = ctx.enter_context(tc.tile_pool(name="zp", bufs=3))
        ps_mm = ctx.enter_context(tc.tile_pool(name="ps_mm", bufs=2, space="PSUM"))
        ps_agg = ctx.enter_context(tc.tile_pool(name="ps_agg", bufs=2, space="PSUM"))
        ps_cs = ctx.enter_context(tc.tile_pool(name="ps_cs", bufs=1, space="PSUM"))
        dram = ctx.enter_context(tc.tile_pool(name="dram", bufs=1, space="DRAM"))

        # ---- persistent / constant tiles ----
        w1_sb = const.tile([P, KIN * hid], dt.bfloat16, tag="w1_sb")
        w2_sb = const.tile([P, KH * hid], dt.bfloat16, tag="w2_sb")
        brow1_sb = const.tile([1, hid], dt.bfloat16, tag="brow1_sb")
        brow2_sb = const.tile([1, hid], dt.bfloat16, tag="brow2_sb")
        iota_sb = const.tile([P, P], dt.bfloat16, tag="iota_sb")
        ident_sb = const.tile([P, P], dt.bfloat16, tag="ident_sb")
        idxlo_sb = const.tile([P, nblk * c_lo * 8], dt.int16, tag="idxlo_sb")
        idxhi_sb = const.tile([P, nblk * c_hi * 8], dt.int16, tag="idxhi_sb")
        dst_sb = const.tile([P, nblk * c_tot], dt.bfloat16, tag="dst_sb")
        dinv_sb = const.tile([P, nblk], dt.float32, tag="dinv_sb")
        binv_sb = const.tile([1, nshard], dt.bfloat16, tag="binv_sb")
        mask_sb = const.tile([P, nblk], dt.bfloat16, tag="mask_sb")

        zT_sb = persist.tile([P, KH * nshard], dt.bfloat16, tag="zT_sb")
        h_sb = persist.tile([P, nblk * hid], dt.bfloat16, tag="h_sb")
        h8_sb = persist.tile([P, nblk * hid], dt.float8e4, tag="h8_sb")

        hshard_d = dram.tile([nshard, hid], dt.float8e4, tag="hshard_d")
        # Shared AG outputs must each have a single writer: one per layer.
        table1_d = dram.tile([npad, hid], dt.float8e4, tag="table1_d",
                             addr_space="Shared")
        table2_d = dram.tile([npad, hid], dt.float8e4, tag="table2_d",
                             addr_space="Shared")
        z1_d = dram.tile([nshard, hid], dt.bfloat16, tag="z1_d")

        # ---- constant loads ----
        nc.sync.dma_start(
            out=w1_sb[:, :].rearrange("p (k f) -> p k f", k=KIN),
            in_=w1[:, :].rearrange("(k p) f -> p k f", p=P))
        nc.sync.dma_start(
            out=w2_sb[:, :].rearrange("p (k f) -> p k f", k=KH),
            in_=w2[:, :].rearrange("(k p) f -> p k f", p=P))
        nc.sync.dma_start(out=brow1_sb[:, :], in_=brow1[:, :])
        nc.sync.dma_start(out=brow2_sb[:, :], in_=brow2[:, :])
        nc.sync.dma_start(out=iota_sb[:, :], in_=iota[:, :])
        nc.sync.dma_start(out=ident_sb[:, :], in_=ident[:, :])
        nc.sync.dma_start(out=idxlo_sb[:, :], in_=idxlo[:, :])
        nc.sync.dma_start(out=idxhi_sb[:, :], in_=idxhi[:, :])
        nc.sync.dma_start(out=dst_sb[:, :], in_=dstf[:, :])
        nc.sync.dma_start(out=dinv_sb[:, :], in_=dinv[:, :])
        nc.sync.dma_start(out=binv_sb[:, :], in_=binv[:, :])
        nc.sync.dma_start(out=mask_sb[:, :], in_=maskc[:, :])

        qcnt = [0]
        first_use = [0]

        def next_q():
            q = qcnt[0] % NQ
            qcnt[0] += 1
            return q

        def dense_layer(lhs_src, w_sb, kc, scale_from_dinv):
            """h_sb[:, nb*hid:...] = scale * (Z @ W) per block (bf16)."""
            for nb in range(nblk):
                ps = ps_mm.tile([P, hid], dt.float32, tag="mm")
                for k in range(kc):
                    nc.tensor.matmul(
                        out=ps[:, :],
                        lhsT=lhs_src(nb, k),
                        rhs=w_sb[:, k * hid:(k + 1) * hid],
                        start=(k == 0), stop=(k == kc - 1))
                scale = dinv_sb[:, nb:nb + 1] if scale_from_dinv else 1.0
                nc.scalar.activation(
                    h_sb[:, nb * hid:(nb + 1) * hid], ps[:, :],
                    mybir.ActivationFunctionType.Copy, scale=scale)
                nc.vector.tensor_copy(
                    out=h8_sb[:, nb * hid:(nb + 1) * hid],
                    in_=h_sb[:, nb * hid:(nb + 1) * hid])

        def distribute(table_d):
            nc.sync.dma_start(
                out=hshard_d[:, :].rearrange("(nb p) f -> p nb f", p=P),
                in_=h8_sb[:, :].rearrange("p (nb f) -> p nb f", nb=nblk))
            nc.gpsimd.collective_compute(
                "AllGather", mybir.AluOpType.bypass,
                replica_groups=[list(range(N_CORES))],
                ins=[hshard_d[:, :].opt()],
                outs=[table_d[:, :].opt()])

        def aggregate(table_d, brow_sb, z_consumer):
            for g0 in range(0, nblk, GB):
                gb = min(GB, nblk - g0)
                mlo = msgp.tile([P, GB * c_lo * hid], dt.float8e4, tag="mlo")
                mhi = msgp.tile([P, GB * c_hi * hid], dt.float8e4, tag="mhi")
                if variant == "noagg":
                    pass
                else:
                    if first_use[0] < MSG_BUFS:
                        # trailing -1 idxs leave pad slots unwritten; zero the
                        # buffers once so stale SBUF can't inject NaNs.
                        first_use[0] += 1
                        nc.vector.memset(mlo[:, :], 0.0)
                        nc.vector.memset(mhi[:, :], 0.0)
                    nc.gpsimd.dma_gather(
                        out_ap=mlo[:, :gb * c_lo * hid]
                            .rearrange("p (c f) -> p c f", c=gb * c_lo),
                        in_ap=table_d[0:half, :],
                        idxs_ap=idxlo_sb[:, g0 * c_lo * 8:(g0 + gb) * c_lo * 8],
                        num_idxs=gb * c_lo * P,
                        num_idxs_reg=gb * c_lo * P,
                        elem_size=hid, single_packet=False,
                        queue_num=next_q())
                    nc.gpsimd.dma_gather(
                        out_ap=mhi[:, :gb * c_hi * hid]
                            .rearrange("p (c f) -> p c f", c=gb * c_hi),
                        in_ap=table_d[half:npad, :],
                        idxs_ap=idxhi_sb[:, g0 * c_hi * 8:(g0 + gb) * c_hi * 8],
                        num_idxs=gb * c_hi * P,
                        num_idxs_reg=gb * c_hi * P,
                        elem_size=hid, single_packet=False,
                        queue_num=next_q())
                for bi in range(gb):
                    nb = g0 + bi
                    agg = ps_agg.tile([P, hid], dt.float32, tag="agg")
                    if variant != "noagg":
                        st = stp.tile([P, c_tot * P], dt.float8e4, tag="st")
                        nc.vector.tensor_tensor(
                            out=st[:, :].rearrange("p (c q) -> p c q", c=c_tot),
                            in0=dst_sb[:, nb * c_tot:(nb + 1) * c_tot]
                                .unsqueeze(2).to_broadcast([P, c_tot, P]),
                            in1=iota_sb[:, :].unsqueeze(1).to_broadcast([P, c_tot, P]),
                            op=mybir.AluOpType.is_equal)
                        for c in range(c_lo):
                            nc.tensor.matmul(
                                out=agg[:, :], lhsT=st[:, c * P:(c + 1) * P],
                                rhs=mlo[:, (bi * c_lo + c) * hid:(bi * c_lo + c + 1) * hid],
                                start=(c == 0), stop=False)
                        for c in range(c_hi):
                            nc.tensor.matmul(
                                out=agg[:, :], lhsT=st[:, (c_lo + c) * P:(c_lo + c + 1) * P],
                                rhs=mhi[:, (bi * c_hi + c) * hid:(bi * c_hi + c + 1) * hid],
                                start=False, stop=False)
                    # bias as K=1 rank-1 update: (1/dinv)_col x bias_row, so the
                    # final dinv scale reconstitutes a plain bias add.
                    nc.tensor.matmul(
                        out=agg[:, :],
                        lhsT=binv_sb[0:1, nb * P:(nb + 1) * P],
                        rhs=brow_sb[0:1, :],
                        start=(variant == "noagg"), stop=False)
                    nc.tensor.matmul(
                        out=agg[:, :], lhsT=ident_sb[:, :],
                        rhs=h_sb[:, nb * hid:(nb + 1) * hid],
                        start=False, stop=True)
                    z = zp.tile([P, hid], dt.bfloat16, tag="z")
                    nc.scalar.activation(
                        z[:, :], agg[:, :], mybir.ActivationFunctionType.Relu,
                        scale=dinv_sb[:, nb:nb + 1])
                    z_consumer(nb, z)

        # ================= layer 1 =================
        XB = 4  # xT blocks per lhs DMA
        xT_tiles = {}

        def xT_lhs(nb, k):
            nbg = nb - nb % XB
            key = (nbg, k)
            if key not in xT_tiles:
                w = min(XB, nblk - nbg)
                t = lhsp.tile([P, XB * P], dt.bfloat16, tag="xTt")
                nc.sync.dma_start(
                    out=t[:, :w * P],
                    in_=xT[k * P:(k + 1) * P, nbg * P:(nbg + w) * P])
                xT_tiles[key] = t
            return xT_tiles[key][:, (nb % XB) * P:(nb % XB + 1) * P]

        dense_layer(xT_lhs, w1_sb, KIN, scale_from_dinv=False)
        distribute(table1_d)

        def z1_write(nb, z):
            nc.sync.dma_start(out=z1_d[nb * P:(nb + 1) * P, :], in_=z[:, :])

        aggregate(table1_d, brow1_sb, z1_write)

        # transpose z1 (DRAM roundtrip, xbar transpose) in one batch so the
        # per-block writes above don't serialize against transpose reads.
        for h in range(KH):
            nc.sync.dma_start(
                out=zT_sb[:, h * nshard:(h + 1) * nshard],
                in_=z1_d[:, h * P:(h + 1) * P],
                transpose=True)

        # ================= layer 2 =================
        def zT_lhs(nb, k):
            return zT_sb[:, k * nshard + nb * P:k * nshard + (nb + 1) * P]

        dense_layer(zT_lhs, w2_sb, KH, scale_from_dinv=True)
        distribute(table2_d)

        cs = [ps_cs.tile([P, 1], dt.float32, tag=f"cs{h}", name=f"cs{h}")
              for h in range(KH)]

        def colsum(nb, z):
            for h in range(KH):
                nc.tensor.matmul(
                    out=cs[h][:, :], lhsT=z[:, h * P:(h + 1) * P],
                    rhs=mask_sb[:, nb:nb + 1],
                    start=(nb == 0), stop=(nb == nblk - 1))

        aggregate(table2_d, brow2_sb, colsum)

        out_sb = zp.tile([P, KH], dt.float32, tag="out_sb")
        for h in range(KH):
            nc.vector.tensor_copy(out=out_sb[:, h:h + 1], in_=cs[h][:, :])
        nc.sync.dma_start(out=out[:, :], in_=out_sb[:, :])

    nc.compile()
    return nc


# --------------------------------------------------------------------------- #
# Entry point
# --------------------------------------------------------------------------- #

_CACHE = {}


def _run(x, edge_index, W1, b1, W2, b2, trace=False):
    from concourse.bass_utils import run_bass_kernel_spmd

    in_maps, meta = _preprocess(x, edge_index, W1, b1, W2, b2)
    key = (meta["nblk"], meta["c_lo"], meta["c_hi"], meta["in_dim"], meta["hid"])
    if key not in _CACHE:
        _CACHE[key] = _build_nc(*key)
    nc = _CACHE[key]
    res = run_bass_kernel_spmd(
        nc, in_maps, core_ids=list(range(N_CORES)), trace=trace)
    parts = [r["partial"] for r in res.results]  # each [P, KH] f32
    colsum = np.sum(np.stack(parts), axis=0)     # [P, KH]
    g = colsum.T.reshape(-1)                     # [hid], g[h*P+p] = colsum[p, h]
    return g / float(meta["n"]), res


def kernel(x, edge_index, W1, b1, W2, b2, Wfc, bfc):
    x = np.asarray(x, dtype=np.float32)
    g, _ = _run(x, edge_index, np.asarray(W1, np.float32), np.asarray(b1, np.float32),
                np.asarray(W2, np.float32), np.asarray(b2, np.float32))
    logits = g.astype(np.float32) @ np.asarray(Wfc, np.float32) + np.asarray(bfc, np.float32)
    return (1.0 / (1.0 + np.exp(-logits))).astype(np.float32)
